# revision 7
# baseline (speedup 1.0000x reference)
"""Self-contained Trainium2 Bass kernel for the GQA attention module.

Sharding: tensor-parallel over heads. Core c owns q-heads [4c..4c+4) and
kv-head c, computes its partial of (attn @ wo); the host sums the 8
partials (the "all-reduce after wo" done host-side during unshard).

Device layout choices:
  - x is passed pre-transposed (xT [DIM, B*S]) so all projections use
    natural weight layouts with no on-device transposes of x.
  - scores are computed transposed (S^T [k, q]) so softmax's P^T is
    directly the moving operand of the PV matmul (no P transpose), and
    the softmax denominator comes free via a ones-column appended to V.
  - RoPE = elementwise muls with host-built cos/sin tables plus a
    pair-swap implemented as a 128x128 permutation matmul.
  - mask tiles are classified host-side: all-zero tiles skip the mask
    add; all-(-inf) tiles skip the scores/exp/PV work entirely.
"""

import sys
import types

sys.path.insert(0, "/opt/trn_rl_repo")

import numpy as np
import ml_dtypes


def _install_axon_hook_shim():
    import antenv

    if "antenv.axon_hooks" in sys.modules:
        return
    m = types.ModuleType("antenv.axon_hooks")
    m._hook = None

    def set_axon_ntff_profile_hook(h):
        m._hook = h

    def get_axon_ntff_profile_hook():
        return m._hook

    m.set_axon_ntff_profile_hook = set_axon_ntff_profile_hook
    m.get_axon_ntff_profile_hook = get_axon_ntff_profile_hook
    sys.modules["antenv.axon_hooks"] = m
    antenv.axon_hooks = m
    try:
        from trn_agent_boot.trn_boot import _ntff_profile_via_ctypes

        hook = _ntff_profile_via_ctypes("/opt/axon/libaxon_pjrt.so")
        if hook is not None:
            m.set_axon_ntff_profile_hook(hook)
    except Exception:
        pass


_install_axon_hook_shim()

import concourse.bass as bass
import concourse.mybir as mybir
import concourse.tile as tile
from concourse.bass_utils import run_bass_kernel_spmd

BF16 = mybir.dt.bfloat16
F32 = mybir.dt.float32

B, S, DIM = 2, 2048, 2048
N_HEADS, N_KV_HEADS, HEAD_DIM = 32, 8, 64
N_CORES = 8
HPC = N_HEADS // N_CORES  # 4 q heads per core
BS = B * S  # 4096 rows
NKT = S // 128  # 16 k tiles per batch
NQC = S // 512  # 4 q chunks per batch
NNT = BS // 512  # 8 projection column blocks
NEG_THRESH = -1e4


def _patched_drain_and_barrier(self, tick_clock, wait_clock):
    # walrus (CoreV3) only accepts one sync-wait on the tile exit drain;
    # split the accumulated waits across single-wait nops.
    nc = self.nc
    drain_inst = nc.sync.drain()
    wait_clock.add_sem_waits(
        drain_inst.ins, tile.ScopedClock({None: tick_clock.global_clock})
    )
    si = drain_inst.ins.sync_info
    sw = list(si.on_wait) if si and si.on_wait else []
    if len(sw) > 1:
        si.on_wait = [sw[0]]
        for w in sw[1:]:
            n2 = nc.sync.nop(nofuse=True)
            if n2.ins.sync_info is None:
                n2.ins.sync_info = mybir.SyncInfo(on_wait=[w], on_update=[])
            else:
                n2.ins.sync_info.on_wait = [w]
    nc.all_engine_barrier()
    assert self.sems is not None
    popped = nc._tile_sem_poison_stack.pop()
    assert popped is self._sem_poison
    nc.clear_and_free_semaphores(list(self.sems.allocated().values()))
    nc.all_engine_barrier()


tile.TileContext._drain_and_barrier = _patched_drain_and_barrier


def _split_multi_waits(nc):
    """walrus (this build) accepts at most one sync-wait per instruction;
    move extra waits onto same-engine nops inserted just before."""
    n_split = 0
    for f in nc.m.functions:
        for blk in f.blocks:
            new_insts = []
            for inst in blk.instructions:
                si = getattr(inst, "sync_info", None)
                if si is not None and si.on_wait and len(si.on_wait) > 1:
                    extra = list(si.on_wait[:-1])
                    si.on_wait = [si.on_wait[-1]]
                    for w in extra:
                        nop = mybir.InstNoOp(
                            name=nc.get_next_instruction_name(), ins=[], outs=[]
                        )
                        nop.engine = inst.engine
                        nop.sync_info = mybir.SyncInfo(on_wait=[w], on_update=[])
                        new_insts.append(nop)
                        n_split += 1
                new_insts.append(inst)
            blk.instructions[:] = new_insts
    return n_split


def build_nc(classes, debug_phase=None):
    """classes[kt][qc] in {'z','n','m'}: mask tile all-zero / all-neg / mixed."""
    nc = bass.Bass("TRN2", target_bir_lowering=False, debug=False, num_devices=N_CORES)

    xT_d = nc.dram_tensor("xT", [DIM, BS], BF16, kind="ExternalInput")
    wq_d = nc.dram_tensor("wq_c", [DIM, HPC * HEAD_DIM], BF16, kind="ExternalInput")
    wk_d = nc.dram_tensor("wk_c", [DIM, HEAD_DIM], BF16, kind="ExternalInput")
    wv_d = nc.dram_tensor("wv_c", [DIM, HEAD_DIM], BF16, kind="ExternalInput")
    wo_d = nc.dram_tensor("wo_c", [HPC * HEAD_DIM, DIM], BF16, kind="ExternalInput")
    maskT_d = nc.dram_tensor("maskT", [S, S], BF16, kind="ExternalInput")
    cosd_d = nc.dram_tensor("cosd", [128, BS], BF16, kind="ExternalInput")
    sind_d = nc.dram_tensor("sind", [128, BS], BF16, kind="ExternalInput")
    perm_d = nc.dram_tensor("perm", [128, 128], BF16, kind="ExternalInput")
    eye64_d = nc.dram_tensor("eye64", [64, 64], BF16, kind="ExternalInput")
    out_d = nc.dram_tensor("out_c", [BS, DIM], F32, kind="ExternalOutput")
    if debug_phase == "proj":
        dbg_q = nc.dram_tensor("dbg_q", [128, 2 * BS], BF16, kind="ExternalOutput")
        dbg_k = nc.dram_tensor("dbg_k", [128, BS], BF16, kind="ExternalOutput")
        dbg_v = nc.dram_tensor("dbg_v", [128, B * NKT * 65], BF16, kind="ExternalOutput")
    if debug_phase == "attn":
        dbg_at = nc.dram_tensor("dbg_at", [128, 2 * BS], BF16, kind="ExternalOutput")

    with tile.TileContext(nc) as tc:
        with (
            tc.tile_pool(name="persist", bufs=1) as persist,
            tc.tile_pool(name="stream", bufs=2) as stream,
            tc.tile_pool(name="small", bufs=3) as small,
        ):
            # ---- persistent tensors ----
            wq_sb = persist.tile([128, NKT, HPC * HEAD_DIM], BF16, tag="wq")
            wk_sb = persist.tile([128, NKT, HEAD_DIM], BF16, tag="wk")
            wv_sb = persist.tile([128, NKT, HEAD_DIM], BF16, tag="wv")
            wo_sb = persist.tile([128, 2, DIM], BF16, tag="wo")
            perm_sb = persist.tile([128, 128], BF16, tag="perm")
            eye64_sb = persist.tile([64, 64], BF16, tag="eye64")
            ones_sb = persist.tile([128, 64], F32, tag="ones")
            q_sb = persist.tile([128, 2, NNT * 512], BF16, tag="q")  # Q^T
            kT_sb = persist.tile([128, BS], BF16, tag="kT")  # K^T (dup halves)
            v_sb = persist.tile([128, B * NKT, 65], BF16, tag="v")  # [V|1]
            at_sb = persist.tile([128, 2, BS], BF16, tag="at")  # A^T

            nc.sync.dma_start(wq_sb[:], wq_d.rearrange("(t p) m -> p t m", p=128))
            nc.sync.dma_start(wk_sb[:], wk_d.rearrange("(t p) m -> p t m", p=128))
            nc.sync.dma_start(wv_sb[:], wv_d.rearrange("(t p) m -> p t m", p=128))
            nc.sync.dma_start(wo_sb[:], wo_d.rearrange("(t p) m -> p t m", p=128))
            nc.sync.dma_start(perm_sb[:], perm_d[:])
            nc.sync.dma_start(eye64_sb[:], eye64_d[:])
            nc.gpsimd.memset(ones_sb[:], 1.0)
            nc.gpsimd.memset(v_sb[:, :, 64:65], 1.0)

            # ---- phase 1: projections + RoPE, per column block of 512 rows ----
            p1 = tc.tile_pool(name="ps_acc", bufs=4, space="PSUM")
            ps_acc = p1.__enter__()
            p1b = tc.tile_pool(name="ps_swp", bufs=2, space="PSUM")
            ps_swp = p1b.__enter__()
            p1c = tc.tile_pool(name="ps_t", bufs=1, space="PSUM")
            ps_t = p1c.__enter__()
            for nt in range(NNT):
                cs = slice(nt * 512, (nt + 1) * 512)
                xblk = stream.tile([128, NKT, 512], BF16, tag="xblk")
                nc.sync.dma_start(
                    xblk[:], xT_d[:, cs].rearrange("(t p) n -> p t n", p=128)
                )
                cosb = stream.tile([128, 512], BF16, tag="cosb")
                sinb = stream.tile([128, 512], BF16, tag="sinb")
                nc.sync.dma_start(cosb[:], cosd_d[:, cs])
                nc.sync.dma_start(sinb[:], sind_d[:, cs])

                # Q projection: 2 M-tiles of 128 rows (= 2 heads each)
                for mt in range(2):
                    psq = ps_acc.tile([128, 512], F32, tag="acc")
                    for kt in range(NKT):
                        nc.tensor.matmul(
                            psq[:],
                            wq_sb[:, kt, mt * 128 : (mt + 1) * 128],
                            xblk[:, kt, :],
                            start=(kt == 0),
                            stop=(kt == NKT - 1),
                        )
                    q_tmp = small.tile([128, 512], BF16, tag="q_tmp")
                    nc.scalar.mul(q_tmp[:], psq[:], 1.0 / 8.0)
                    psw = ps_swp.tile([128, 512], F32, tag="swp")
                    nc.tensor.matmul(psw[:], perm_sb[:], q_tmp[:])
                    v1 = small.tile([128, 512], BF16, tag="v1")
                    nc.vector.tensor_mul(v1[:], q_tmp[:], cosb[:])
                    v2 = small.tile([128, 512], BF16, tag="v2")
                    nc.vector.tensor_mul(v2[:], psw[:], sinb[:])
                    nc.vector.tensor_add(q_sb[:, mt, cs], v1[:], v2[:])

                # K projection (single 64-row tile)
                psk = ps_acc.tile([64, 512], F32, tag="acc")
                for kt in range(NKT):
                    nc.tensor.matmul(
                        psk[:],
                        wk_sb[:, kt, :],
                        xblk[:, kt, :],
                        start=(kt == 0),
                        stop=(kt == NKT - 1),
                    )
                k_tmp = small.tile([64, 512], BF16, tag="k_tmp")
                nc.scalar.copy(k_tmp[:], psk[:])
                pskw = ps_swp.tile([64, 512], F32, tag="swp")
                nc.tensor.matmul(pskw[:], perm_sb[0:64, 0:64], k_tmp[:])
                kv1 = small.tile([64, 512], BF16, tag="kv1")
                nc.vector.tensor_mul(kv1[:], k_tmp[:], cosb[0:64, :])
                kv2 = small.tile([64, 512], BF16, tag="kv2")
                nc.vector.tensor_mul(kv2[:], pskw[:], sinb[0:64, :])
                nc.vector.tensor_add(kT_sb[0:64, cs], kv1[:], kv2[:])
                # duplicate K^T into partitions 64..127 (so odd q-heads can
                # use it as lhsT at their partition base)
                nc.sync.dma_start(kT_sb[64:128, cs], kT_sb[0:64, cs])

                # V projection -> V^T [64, 512], then transpose to natural V
                psv = ps_acc.tile([64, 512], F32, tag="acc")
                for kt in range(NKT):
                    nc.tensor.matmul(
                        psv[:],
                        wv_sb[:, kt, :],
                        xblk[:, kt, :],
                        start=(kt == 0),
                        stop=(kt == NKT - 1),
                    )
                v_tmp = small.tile([64, 512], BF16, tag="v_tmp")
                nc.scalar.copy(v_tmp[:], psv[:])
                for j in range(4):
                    pst = ps_t.tile([128, 64], BF16, tag="pst")
                    nc.tensor.transpose(
                        pst[:], v_tmp[:, j * 128 : (j + 1) * 128], eye64_sb[:]
                    )
                    rc = nt * 4 + j
                    nc.scalar.copy(v_sb[:, rc, 0:64], pst[:])

            p1c.__exit__(None, None, None)
            p1b.__exit__(None, None, None)
            p1.__exit__(None, None, None)

            if debug_phase == "proj":
                nc.sync.dma_start(dbg_q[:], q_sb[:].rearrange("p a b -> p (a b)"))
                nc.sync.dma_start(dbg_k[:], kT_sb[:])
                nc.sync.dma_start(dbg_v[:], v_sb[:].rearrange("p a b -> p (a b)"))

            # ---- phase 2: attention per (batch, local head, q chunk) ----
            p2 = tc.tile_pool(name="ps_s", bufs=3, space="PSUM")
            ps_s = p2.__enter__()
            p2b = tc.tile_pool(name="ps_o", bufs=2, space="PSUM")
            ps_o = p2b.__enter__()
            p2c = tc.tile_pool(name="ps_b", bufs=2, space="PSUM")
            ps_b = p2c.__enter__()
            for b in range(B) if debug_phase != "proj" else []:
                for h in range(HPC):
                    hb = (h % 2) * 64
                    mt = h // 2
                    for qc in range(NQC):
                        qs = slice(b * S + qc * 512, b * S + (qc + 1) * 512)
                        acts = [kt for kt in range(NKT) if classes[kt][qc] != "n"]
                        assert acts, "fully-masked q chunk unsupported"
                        pso = ps_o.tile([65, 512], F32, tag="pso")
                        for kt in acts:
                            ks = slice(b * S + kt * 128, b * S + (kt + 1) * 128)
                            pss = ps_s.tile([128, 512], F32, tag="pss")
                            nc.tensor.matmul(
                                pss[:],
                                kT_sb[hb : hb + 64, ks],
                                q_sb[hb : hb + 64, mt, qs],
                            )
                            p_t = small.tile([128, 512], BF16, tag="p_t")
                            if classes[kt][qc] == "z":
                                nc.scalar.activation(
                                    p_t[:], pss[:], mybir.ActivationFunctionType.Exp
                                )
                            else:
                                mk = small.tile([128, 512], BF16, tag="mk")
                                nc.sync.dma_start(
                                    mk[:],
                                    maskT_d[
                                        kt * 128 : (kt + 1) * 128,
                                        qc * 512 : (qc + 1) * 512,
                                    ],
                                )
                                s2 = small.tile([128, 512], BF16, tag="s2")
                                nc.vector.tensor_add(s2[:], pss[:], mk[:])
                                nc.scalar.activation(
                                    p_t[:], s2[:], mybir.ActivationFunctionType.Exp
                                )
                            nc.tensor.matmul(
                                pso[:],
                                v_sb[:, b * NKT + kt, :],
                                p_t[:],
                                start=(kt == acts[0]),
                                stop=(kt == acts[-1]),
                            )
                        # normalize: rows 0..63 = unnormalized O^T, row 64 = denom
                        rec = small.tile([65, 512], F32, tag="rec")
                        nc.vector.reciprocal(rec[64:65, :], pso[64:65, :])
                        psb2 = ps_b.tile([64, 512], F32, tag="psb2")
                        nc.tensor.matmul(psb2[:], ones_sb[64:65, :], rec[64:65, :])
                        bc = small.tile([64, 512], BF16, tag="bc")
                        nc.scalar.copy(bc[:], psb2[:])
                        if hb == 0:
                            nc.vector.tensor_mul(
                                at_sb[0:64, mt, qs], pso[0:64, :], bc[:]
                            )
                        else:
                            att = small.tile([64, 512], BF16, tag="att")
                            nc.vector.tensor_mul(att[:], pso[0:64, :], bc[:])
                            nc.sync.dma_start(at_sb[64:128, mt, qs], att[:])

            p2c.__exit__(None, None, None)
            p2b.__exit__(None, None, None)
            p2.__exit__(None, None, None)

            if debug_phase == "attn":
                nc.sync.dma_start(dbg_at[:], at_sb[:].rearrange("p a b -> p (a b)"))

            # ---- phase 3: output projection (partial; host sums cores) ----
            p3 = tc.tile_pool(name="ps_d", bufs=4, space="PSUM")
            ps_d = p3.__enter__()
            for mt in range(BS // 128) if debug_phase is None else []:
                for ntc in range(4):
                    psd = ps_d.tile([128, 512], F32, tag="psd")
                    for ch in range(2):
                        nc.tensor.matmul(
                            psd[:],
                            at_sb[:, ch, mt * 128 : (mt + 1) * 128],
                            wo_sb[:, ch, ntc * 512 : (ntc + 1) * 512],
                            start=(ch == 0),
                            stop=(ch == 1),
                        )
                    ot = small.tile([128, 512], F32, tag="ot")
                    nc.scalar.copy(ot[:], psd[:])
                    nc.sync.dma_start(
                        out_d[mt * 128 : (mt + 1) * 128, ntc * 512 : (ntc + 1) * 512],
                        ot[:],
                    )
            p3.__exit__(None, None, None)
    _split_multi_waits(nc)
    return nc


_NC_CACHE = {}


def _classify_mask(mask):
    """Per (kt, qc) tile class from the [S, S] additive mask ([q, k])."""
    classes = []
    for kt in range(NKT):
        row = []
        for qc in range(NQC):
            sub = mask[qc * 512 : (qc + 1) * 512, kt * 128 : (kt + 1) * 128]
            if np.all(sub == 0.0):
                row.append("z")
            elif np.all(sub <= NEG_THRESH):
                row.append("n")
            else:
                row.append("m")
        classes.append(row)
    # every q row must keep at least one active k tile
    for qc in range(NQC):
        if all(classes[kt][qc] == "n" for kt in range(NKT)):
            for kt in range(NKT):
                if classes[kt][qc] == "n":
                    classes[kt][qc] = "m"
    return classes


def _prep_inputs(x, freqs_cos, freqs_sin, mask, wq, wk, wv, wo):
    bf = ml_dtypes.bfloat16
    x2 = np.ascontiguousarray(np.asarray(x, dtype=np.float32).reshape(BS, DIM))
    xT = np.ascontiguousarray(x2.T).astype(bf)
    maskT = np.ascontiguousarray(np.asarray(mask, dtype=np.float32).T).astype(bf)

    cos = np.asarray(freqs_cos, dtype=np.float32)  # [S, 32]
    sin = np.asarray(freqs_sin, dtype=np.float32)
    # cosd[d, b*S+s] = cos[s, (d%64)//2]; sind alternates -sin/+sin
    d = np.arange(128)
    pair = (d % 64) // 2
    cosd = cos[:, pair].T  # [128, S]
    sgn = np.where(d % 2 == 0, -1.0, 1.0).astype(np.float32)
    sind = sin[:, pair].T * sgn[:, None]
    cosd = np.ascontiguousarray(np.tile(cosd, (1, B))).astype(bf)
    sind = np.ascontiguousarray(np.tile(sind, (1, B))).astype(bf)

    perm = np.zeros((128, 128), dtype=np.float32)
    idx = np.arange(128)
    perm[idx ^ 1, idx] = 1.0
    perm = perm.astype(bf)
    eye64 = np.eye(64, dtype=np.float32).astype(bf)

    wq = np.asarray(wq, dtype=np.float32)
    wk = np.asarray(wk, dtype=np.float32)
    wv = np.asarray(wv, dtype=np.float32)
    wo = np.asarray(wo, dtype=np.float32)

    in_maps = []
    for c in range(N_CORES):
        hs = slice(c * HPC * HEAD_DIM, (c + 1) * HPC * HEAD_DIM)
        ks = slice(c * HEAD_DIM, (c + 1) * HEAD_DIM)
        in_maps.append(
            {
                "xT": xT,
                "wq_c": np.ascontiguousarray(wq[:, hs]).astype(bf),
                "wk_c": np.ascontiguousarray(wk[:, ks]).astype(bf),
                "wv_c": np.ascontiguousarray(wv[:, ks]).astype(bf),
                "wo_c": np.ascontiguousarray(wo[hs, :]).astype(bf),
                "maskT": maskT,
                "cosd": cosd,
                "sind": sind,
                "perm": perm,
                "eye64": eye64,
            }
        )
    return in_maps


def kernel(x, freqs_cos, freqs_sin, mask, wq, wk, wv, wo, _trace=False):
    classes = _classify_mask(np.asarray(mask, dtype=np.float32))
    key = tuple(tuple(r) for r in classes)
    if key not in _NC_CACHE:
        _NC_CACHE[key] = build_nc(classes)
    nc = _NC_CACHE[key]
    in_maps = _prep_inputs(x, freqs_cos, freqs_sin, mask, wq, wk, wv, wo)
    res = run_bass_kernel_spmd(
        nc, in_maps, core_ids=list(range(N_CORES)), trace=_trace
    )
    out = np.zeros((BS, DIM), dtype=np.float32)
    for c in range(N_CORES):
        out += res.results[c]["out_c"]
    out = out.reshape(B, S, DIM)
    if _trace:
        kernel._last_exec_time_ns = res.exec_time_ns
        kernel._last_profile_json = res.profile_json
    return out


# revision 8
# speedup vs baseline: 1.2082x; 1.2082x over previous
"""Self-contained Trainium2 Bass kernel for the GQA attention module.

Sharding: tensor-parallel over heads. Core c owns q-heads [4c..4c+4) and
kv-head c, computes its partial of (attn @ wo); the host sums the 8
partials (the "all-reduce after wo" done host-side during unshard).

Device layout choices:
  - x is passed pre-transposed (xT [DIM, B*S]) so all projections use
    natural weight layouts with no on-device transposes of x.
  - scores are computed transposed (S^T [k, q]) so softmax's P^T is
    directly the moving operand of the PV matmul (no P transpose), and
    the softmax denominator comes free via a ones-column appended to V.
  - RoPE = elementwise muls with host-built cos/sin tables plus a
    pair-swap implemented as a 128x128 permutation matmul.
  - mask tiles are classified host-side: all-zero tiles skip the mask
    add; all-(-inf) tiles skip the scores/exp/PV work entirely.
"""

import sys
import types

sys.path.insert(0, "/opt/trn_rl_repo")

import numpy as np
import ml_dtypes


def _install_axon_hook_shim():
    import antenv

    if "antenv.axon_hooks" in sys.modules:
        return
    m = types.ModuleType("antenv.axon_hooks")
    m._hook = None

    def set_axon_ntff_profile_hook(h):
        m._hook = h

    def get_axon_ntff_profile_hook():
        return m._hook

    m.set_axon_ntff_profile_hook = set_axon_ntff_profile_hook
    m.get_axon_ntff_profile_hook = get_axon_ntff_profile_hook
    sys.modules["antenv.axon_hooks"] = m
    antenv.axon_hooks = m
    try:
        from trn_agent_boot.trn_boot import _ntff_profile_via_ctypes

        hook = _ntff_profile_via_ctypes("/opt/axon/libaxon_pjrt.so")
        if hook is not None:
            m.set_axon_ntff_profile_hook(hook)
    except Exception:
        pass


_install_axon_hook_shim()

import concourse.bass as bass
import concourse.mybir as mybir
import concourse.tile as tile
from concourse.bass_utils import run_bass_kernel_spmd

BF16 = mybir.dt.bfloat16
F32 = mybir.dt.float32

B, S, DIM = 2, 2048, 2048
N_HEADS, N_KV_HEADS, HEAD_DIM = 32, 8, 64
N_CORES = 8
HPC = N_HEADS // N_CORES  # 4 q heads per core
BS = B * S  # 4096 rows
NKT = S // 128  # 16 k tiles per batch
NQC = S // 512  # 4 q chunks per batch
NNT = BS // 512  # 8 projection column blocks
NEG_THRESH = -1e4


def _patched_drain_and_barrier(self, tick_clock, wait_clock):
    # walrus (CoreV3) only accepts one sync-wait on the tile exit drain;
    # split the accumulated waits across single-wait nops.
    nc = self.nc
    drain_inst = nc.sync.drain()
    wait_clock.add_sem_waits(
        drain_inst.ins, tile.ScopedClock({None: tick_clock.global_clock})
    )
    si = drain_inst.ins.sync_info
    sw = list(si.on_wait) if si and si.on_wait else []
    if len(sw) > 1:
        si.on_wait = [sw[0]]
        for w in sw[1:]:
            n2 = nc.sync.nop(nofuse=True)
            if n2.ins.sync_info is None:
                n2.ins.sync_info = mybir.SyncInfo(on_wait=[w], on_update=[])
            else:
                n2.ins.sync_info.on_wait = [w]
    nc.all_engine_barrier()
    assert self.sems is not None
    popped = nc._tile_sem_poison_stack.pop()
    assert popped is self._sem_poison
    nc.clear_and_free_semaphores(list(self.sems.allocated().values()))
    nc.all_engine_barrier()


tile.TileContext._drain_and_barrier = _patched_drain_and_barrier


def _split_multi_waits(nc):
    """walrus (this build) accepts at most one sync-wait per instruction;
    move extra waits onto same-engine nops inserted just before."""
    n_split = 0
    for f in nc.m.functions:
        for blk in f.blocks:
            new_insts = []
            for inst in blk.instructions:
                si = getattr(inst, "sync_info", None)
                if si is not None and si.on_wait and len(si.on_wait) > 1:
                    extra = list(si.on_wait[:-1])
                    si.on_wait = [si.on_wait[-1]]
                    for w in extra:
                        nop = mybir.InstNoOp(
                            name=nc.get_next_instruction_name(), ins=[], outs=[]
                        )
                        nop.engine = inst.engine
                        nop.sync_info = mybir.SyncInfo(on_wait=[w], on_update=[])
                        new_insts.append(nop)
                        n_split += 1
                new_insts.append(inst)
            blk.instructions[:] = new_insts
    return n_split


def build_nc(classes, debug_phase=None):
    """classes[kt][qc] in {'z','n','m'}: mask tile all-zero / all-neg / mixed."""
    nc = bass.Bass("TRN2", target_bir_lowering=False, debug=False, num_devices=N_CORES)

    xT_d = nc.dram_tensor("xT", [DIM, BS], BF16, kind="ExternalInput")
    wq_d = nc.dram_tensor("wq_c", [DIM, HPC * HEAD_DIM], BF16, kind="ExternalInput")
    wk_d = nc.dram_tensor("wk_c", [DIM, HEAD_DIM], BF16, kind="ExternalInput")
    wv_d = nc.dram_tensor("wv_c", [DIM, HEAD_DIM], BF16, kind="ExternalInput")
    wo_d = nc.dram_tensor("wo_c", [HPC * HEAD_DIM, DIM], BF16, kind="ExternalInput")
    maskT_d = nc.dram_tensor("maskT", [S, S], BF16, kind="ExternalInput")
    cosd_d = nc.dram_tensor("cosd", [128, BS], BF16, kind="ExternalInput")
    sind_d = nc.dram_tensor("sind", [128, BS], BF16, kind="ExternalInput")
    perm_d = nc.dram_tensor("perm", [128, 128], BF16, kind="ExternalInput")
    eye64_d = nc.dram_tensor("eye64", [64, 64], BF16, kind="ExternalInput")
    out_d = nc.dram_tensor("out_c", [BS, DIM], BF16, kind="ExternalOutput")
    if debug_phase == "proj":
        dbg_q = nc.dram_tensor("dbg_q", [128, 2 * BS], BF16, kind="ExternalOutput")
        dbg_k = nc.dram_tensor("dbg_k", [128, BS], BF16, kind="ExternalOutput")
        dbg_v = nc.dram_tensor("dbg_v", [128, B * NKT * 65], BF16, kind="ExternalOutput")
    if debug_phase == "attn":
        dbg_at = nc.dram_tensor("dbg_at", [128, 2 * BS], BF16, kind="ExternalOutput")

    with tile.TileContext(nc) as tc:
        with (
            tc.tile_pool(name="persist", bufs=1) as persist,
            tc.tile_pool(name="stream", bufs=2) as stream,
            tc.tile_pool(name="small", bufs=3) as small,
        ):
            # ---- persistent tensors ----
            wq_sb = persist.tile([128, NKT, HPC * HEAD_DIM], BF16, tag="wq")
            wk_sb = persist.tile([128, NKT, HEAD_DIM], BF16, tag="wk")
            wv_sb = persist.tile([128, NKT, HEAD_DIM], BF16, tag="wv")
            wo_sb = persist.tile([128, 2, DIM], BF16, tag="wo")
            perm_sb = persist.tile([128, 128], BF16, tag="perm")
            eye64_sb = persist.tile([64, 64], BF16, tag="eye64")
            ones_sb = persist.tile([128, 64], BF16, tag="ones")
            q_sb = persist.tile([128, 2, NNT * 512], BF16, tag="q")  # Q^T
            kT_sb = persist.tile([128, BS], BF16, tag="kT")  # K^T (dup halves)
            v_sb = persist.tile([128, B * NKT, 65], BF16, tag="v")  # [V|1]
            at_sb = persist.tile([128, 2, BS], BF16, tag="at")  # A^T

            nc.sync.dma_start(wq_sb[:], wq_d.rearrange("(t p) m -> p t m", p=128))
            nc.sync.dma_start(wk_sb[:], wk_d.rearrange("(t p) m -> p t m", p=128))
            nc.sync.dma_start(wv_sb[:], wv_d.rearrange("(t p) m -> p t m", p=128))
            nc.sync.dma_start(wo_sb[:], wo_d.rearrange("(t p) m -> p t m", p=128))
            nc.sync.dma_start(perm_sb[:], perm_d[:])
            nc.sync.dma_start(eye64_sb[:], eye64_d[:])
            nc.gpsimd.memset(ones_sb[:], 1.0)
            nc.gpsimd.memset(v_sb[:, :, 64:65], 1.0)

            # ---- phase 1: projections + RoPE, per column block of 512 rows ----
            p1 = tc.tile_pool(name="ps_acc", bufs=4, space="PSUM")
            ps_acc = p1.__enter__()
            p1b = tc.tile_pool(name="ps_swp", bufs=2, space="PSUM")
            ps_swp = p1b.__enter__()
            p1c = tc.tile_pool(name="ps_t", bufs=1, space="PSUM")
            ps_t = p1c.__enter__()
            for nt in range(NNT):
                cs = slice(nt * 512, (nt + 1) * 512)
                xblk = stream.tile([128, NKT, 512], BF16, tag="xblk")
                nc.sync.dma_start(
                    xblk[:], xT_d[:, cs].rearrange("(t p) n -> p t n", p=128)
                )
                cosb = stream.tile([128, 512], BF16, tag="cosb")
                sinb = stream.tile([128, 512], BF16, tag="sinb")
                nc.sync.dma_start(cosb[:], cosd_d[:, cs])
                nc.sync.dma_start(sinb[:], sind_d[:, cs])

                # Q projection: 2 M-tiles of 128 rows (= 2 heads each)
                for mt in range(2):
                    psq = ps_acc.tile([128, 512], F32, tag="acc")
                    for kt in range(NKT):
                        nc.tensor.matmul(
                            psq[:],
                            wq_sb[:, kt, mt * 128 : (mt + 1) * 128],
                            xblk[:, kt, :],
                            start=(kt == 0),
                            stop=(kt == NKT - 1),
                        )
                    q_tmp = small.tile([128, 512], BF16, tag="q_tmp")
                    nc.scalar.mul(q_tmp[:], psq[:], 1.0 / 8.0)
                    psw = ps_swp.tile([128, 512], F32, tag="swp")
                    nc.tensor.matmul(psw[:], perm_sb[:], q_tmp[:])
                    v1 = small.tile([128, 512], BF16, tag="v1")
                    nc.vector.tensor_mul(v1[:], q_tmp[:], cosb[:])
                    v2 = small.tile([128, 512], BF16, tag="v2")
                    nc.vector.tensor_mul(v2[:], psw[:], sinb[:])
                    nc.vector.tensor_add(q_sb[:, mt, cs], v1[:], v2[:])

                # K projection (single 64-row tile)
                psk = ps_acc.tile([64, 512], F32, tag="acc")
                for kt in range(NKT):
                    nc.tensor.matmul(
                        psk[:],
                        wk_sb[:, kt, :],
                        xblk[:, kt, :],
                        start=(kt == 0),
                        stop=(kt == NKT - 1),
                    )
                k_tmp = small.tile([64, 512], BF16, tag="k_tmp")
                nc.scalar.copy(k_tmp[:], psk[:])
                pskw = ps_swp.tile([64, 512], F32, tag="swp")
                nc.tensor.matmul(pskw[:], perm_sb[0:64, 0:64], k_tmp[:])
                kv1 = small.tile([64, 512], BF16, tag="kv1")
                nc.vector.tensor_mul(kv1[:], k_tmp[:], cosb[0:64, :])
                kv2 = small.tile([64, 512], BF16, tag="kv2")
                nc.vector.tensor_mul(kv2[:], pskw[:], sinb[0:64, :])
                nc.vector.tensor_add(kT_sb[0:64, cs], kv1[:], kv2[:])
                # duplicate K^T into partitions 64..127 (so odd q-heads can
                # use it as lhsT at their partition base)
                nc.sync.dma_start(kT_sb[64:128, cs], kT_sb[0:64, cs])

                # V projection -> V^T [64, 512], then transpose to natural V
                psv = ps_acc.tile([64, 512], F32, tag="acc")
                for kt in range(NKT):
                    nc.tensor.matmul(
                        psv[:],
                        wv_sb[:, kt, :],
                        xblk[:, kt, :],
                        start=(kt == 0),
                        stop=(kt == NKT - 1),
                    )
                v_tmp = small.tile([64, 512], BF16, tag="v_tmp")
                nc.scalar.copy(v_tmp[:], psv[:])
                for j in range(4):
                    pst = ps_t.tile([128, 64], BF16, tag="pst")
                    nc.tensor.transpose(
                        pst[:], v_tmp[:, j * 128 : (j + 1) * 128], eye64_sb[:]
                    )
                    rc = nt * 4 + j
                    nc.scalar.copy(v_sb[:, rc, 0:64], pst[:])

            p1c.__exit__(None, None, None)
            p1b.__exit__(None, None, None)
            p1.__exit__(None, None, None)

            if debug_phase == "proj":
                nc.sync.dma_start(dbg_q[:], q_sb[:].rearrange("p a b -> p (a b)"))
                nc.sync.dma_start(dbg_k[:], kT_sb[:])
                nc.sync.dma_start(dbg_v[:], v_sb[:].rearrange("p a b -> p (a b)"))

            # ---- phase 2+3 fused: attention + wo slice per (batch, q chunk) ----
            p2 = tc.tile_pool(name="ps_s", bufs=3, space="PSUM")
            ps_s = p2.__enter__()
            p2b = tc.tile_pool(name="ps_o", bufs=2, space="PSUM")
            ps_o = p2b.__enter__()
            p2c = tc.tile_pool(name="ps_b", bufs=1, space="PSUM")
            ps_b = p2c.__enter__()
            p3 = tc.tile_pool(name="ps_d", bufs=2, space="PSUM")
            ps_d = p3.__enter__()
            for b in range(B) if debug_phase != "proj" else []:
                for qc in range(NQC):
                    qs = slice(b * S + qc * 512, b * S + (qc + 1) * 512)
                    acts = [kt for kt in range(NKT) if classes[kt][qc] != "n"]
                    assert acts, "fully-masked q chunk unsupported"
                    for h in range(HPC):
                        hb = (h % 2) * 64
                        mt = h // 2
                        pso = ps_o.tile([65, 512], F32, tag="pso")
                        for kt in acts:
                            ks = slice(b * S + kt * 128, b * S + (kt + 1) * 128)
                            pss = ps_s.tile([128, 512], F32, tag="pss")
                            nc.tensor.matmul(
                                pss[:],
                                kT_sb[hb : hb + 64, ks],
                                q_sb[hb : hb + 64, mt, qs],
                            )
                            p_t = small.tile([128, 512], BF16, tag="p_t")
                            if classes[kt][qc] == "z":
                                nc.scalar.activation(
                                    p_t[:], pss[:], mybir.ActivationFunctionType.Exp
                                )
                            else:
                                mk = small.tile([128, 512], BF16, tag="mk")
                                nc.sync.dma_start(
                                    mk[:],
                                    maskT_d[
                                        kt * 128 : (kt + 1) * 128,
                                        qc * 512 : (qc + 1) * 512,
                                    ],
                                )
                                s2 = small.tile([128, 512], BF16, tag="s2")
                                nc.vector.tensor_add(s2[:], pss[:], mk[:])
                                nc.scalar.activation(
                                    p_t[:], s2[:], mybir.ActivationFunctionType.Exp
                                )
                            nc.tensor.matmul(
                                pso[:],
                                v_sb[:, b * NKT + kt, :],
                                p_t[:],
                                start=(kt == acts[0]),
                                stop=(kt == acts[-1]),
                            )
                        # normalize: rows 0..63 = unnormalized O^T, row 64 = denom
                        rec = small.tile([65, 512], BF16, tag="rec")
                        with nc.allow_low_precision(reason="softmax denom bf16"):
                            nc.vector.reciprocal(rec[64:65, :], pso[64:65, :])
                        psb2 = ps_b.tile([64, 512], F32, tag="psb2")
                        nc.tensor.matmul(psb2[:], ones_sb[64:65, :], rec[64:65, :])
                        bc = small.tile([64, 512], BF16, tag="bc")
                        nc.scalar.copy(bc[:], psb2[:])
                        if hb == 0:
                            nc.vector.tensor_mul(
                                at_sb[0:64, mt, qs], pso[0:64, :], bc[:]
                            )
                        else:
                            att = small.tile([64, 512], BF16, tag="att")
                            nc.vector.tensor_mul(att[:], pso[0:64, :], bc[:])
                            nc.sync.dma_start(at_sb[64:128, mt, qs], att[:])
                    if debug_phase == "attn":
                        continue
                    # wo slice for these 512 rows (4 row-tiles of 128)
                    for j in range(4):
                        mt2 = (b * S + qc * 512) // 128 + j
                        for ntc in range(4):
                            psd = ps_d.tile([128, 512], F32, tag="psd")
                            for ch in range(2):
                                nc.tensor.matmul(
                                    psd[:],
                                    at_sb[:, ch, mt2 * 128 : (mt2 + 1) * 128],
                                    wo_sb[:, ch, ntc * 512 : (ntc + 1) * 512],
                                    start=(ch == 0),
                                    stop=(ch == 1),
                                )
                            ot = small.tile([128, 512], BF16, tag="ot")
                            nc.any.tensor_copy(ot[:], psd[:])
                            nc.sync.dma_start(
                                out_d[
                                    mt2 * 128 : (mt2 + 1) * 128,
                                    ntc * 512 : (ntc + 1) * 512,
                                ],
                                ot[:],
                            )

            if debug_phase == "attn":
                nc.sync.dma_start(dbg_at[:], at_sb[:].rearrange("p a b -> p (a b)"))
            p3.__exit__(None, None, None)
            p2c.__exit__(None, None, None)
            p2b.__exit__(None, None, None)
            p2.__exit__(None, None, None)
    _split_multi_waits(nc)
    return nc


_NC_CACHE = {}


def _classify_mask(mask):
    """Per (kt, qc) tile class from the [S, S] additive mask ([q, k])."""
    classes = []
    for kt in range(NKT):
        row = []
        for qc in range(NQC):
            sub = mask[qc * 512 : (qc + 1) * 512, kt * 128 : (kt + 1) * 128]
            if np.all(sub == 0.0):
                row.append("z")
            elif np.all(sub <= NEG_THRESH):
                row.append("n")
            else:
                row.append("m")
        classes.append(row)
    # every q row must keep at least one active k tile
    for qc in range(NQC):
        if all(classes[kt][qc] == "n" for kt in range(NKT)):
            for kt in range(NKT):
                if classes[kt][qc] == "n":
                    classes[kt][qc] = "m"
    return classes


def _prep_inputs(x, freqs_cos, freqs_sin, mask, wq, wk, wv, wo):
    bf = ml_dtypes.bfloat16
    x2 = np.ascontiguousarray(np.asarray(x, dtype=np.float32).reshape(BS, DIM))
    xT = np.ascontiguousarray(x2.T).astype(bf)
    maskT = np.ascontiguousarray(np.asarray(mask, dtype=np.float32).T).astype(bf)

    cos = np.asarray(freqs_cos, dtype=np.float32)  # [S, 32]
    sin = np.asarray(freqs_sin, dtype=np.float32)
    # cosd[d, b*S+s] = cos[s, (d%64)//2]; sind alternates -sin/+sin
    d = np.arange(128)
    pair = (d % 64) // 2
    cosd = cos[:, pair].T  # [128, S]
    sgn = np.where(d % 2 == 0, -1.0, 1.0).astype(np.float32)
    sind = sin[:, pair].T * sgn[:, None]
    cosd = np.ascontiguousarray(np.tile(cosd, (1, B))).astype(bf)
    sind = np.ascontiguousarray(np.tile(sind, (1, B))).astype(bf)

    perm = np.zeros((128, 128), dtype=np.float32)
    idx = np.arange(128)
    perm[idx ^ 1, idx] = 1.0
    perm = perm.astype(bf)
    eye64 = np.eye(64, dtype=np.float32).astype(bf)

    wq = np.asarray(wq, dtype=np.float32)
    wk = np.asarray(wk, dtype=np.float32)
    wv = np.asarray(wv, dtype=np.float32)
    wo = np.asarray(wo, dtype=np.float32)

    in_maps = []
    for c in range(N_CORES):
        hs = slice(c * HPC * HEAD_DIM, (c + 1) * HPC * HEAD_DIM)
        ks = slice(c * HEAD_DIM, (c + 1) * HEAD_DIM)
        in_maps.append(
            {
                "xT": xT,
                "wq_c": np.ascontiguousarray(wq[:, hs]).astype(bf),
                "wk_c": np.ascontiguousarray(wk[:, ks]).astype(bf),
                "wv_c": np.ascontiguousarray(wv[:, ks]).astype(bf),
                "wo_c": np.ascontiguousarray(wo[hs, :]).astype(bf),
                "maskT": maskT,
                "cosd": cosd,
                "sind": sind,
                "perm": perm,
                "eye64": eye64,
            }
        )
    return in_maps


def kernel(x, freqs_cos, freqs_sin, mask, wq, wk, wv, wo, _trace=False):
    classes = _classify_mask(np.asarray(mask, dtype=np.float32))
    key = tuple(tuple(r) for r in classes)
    if key not in _NC_CACHE:
        _NC_CACHE[key] = build_nc(classes)
    nc = _NC_CACHE[key]
    in_maps = _prep_inputs(x, freqs_cos, freqs_sin, mask, wq, wk, wv, wo)
    res = run_bass_kernel_spmd(
        nc, in_maps, core_ids=list(range(N_CORES)), trace=_trace
    )
    out = np.zeros((BS, DIM), dtype=np.float32)
    for c in range(N_CORES):
        out += np.asarray(res.results[c]["out_c"], dtype=np.float32)
    out = out.reshape(B, S, DIM)
    if _trace:
        kernel._last_exec_time_ns = res.exec_time_ns
        kernel._last_profile_json = res.profile_json
    return out


# revision 12
# speedup vs baseline: 1.2662x; 1.0480x over previous
"""Self-contained Trainium2 Bass kernel for the GQA attention module.

Sharding: tensor-parallel over heads. Core c owns q-heads [4c..4c+4) and
kv-head c, computes its partial of (attn @ wo); the host sums the 8
partials (the "all-reduce after wo" done host-side during unshard).

Device layout choices:
  - x is passed pre-transposed (xT [DIM, B*S]) so all projections use
    natural weight layouts with no on-device transposes of x.
  - scores are computed transposed (S^T [k, q]) so softmax's P^T is
    directly the moving operand of the PV matmul (no P transpose), and
    the softmax denominator comes free via a ones-column appended to V.
  - RoPE = elementwise muls with host-built cos/sin tables plus a
    pair-swap implemented as a 128x128 permutation matmul.
  - mask tiles are classified host-side: all-zero tiles skip the mask
    add; all-(-inf) tiles skip the scores/exp/PV work entirely.
"""

import sys
import types

sys.path.insert(0, "/opt/trn_rl_repo")

import numpy as np
import ml_dtypes


def _install_axon_hook_shim():
    import antenv

    if "antenv.axon_hooks" in sys.modules:
        return
    m = types.ModuleType("antenv.axon_hooks")
    m._hook = None

    def set_axon_ntff_profile_hook(h):
        m._hook = h

    def get_axon_ntff_profile_hook():
        return m._hook

    m.set_axon_ntff_profile_hook = set_axon_ntff_profile_hook
    m.get_axon_ntff_profile_hook = get_axon_ntff_profile_hook
    sys.modules["antenv.axon_hooks"] = m
    antenv.axon_hooks = m
    try:
        from trn_agent_boot.trn_boot import _ntff_profile_via_ctypes

        hook = _ntff_profile_via_ctypes("/opt/axon/libaxon_pjrt.so")
        if hook is not None:
            m.set_axon_ntff_profile_hook(hook)
    except Exception:
        pass


_install_axon_hook_shim()

import concourse.bass as bass
import concourse.mybir as mybir
import concourse.tile as tile
from concourse.bass_utils import run_bass_kernel_spmd

BF16 = mybir.dt.bfloat16
F32 = mybir.dt.float32

B, S, DIM = 2, 2048, 2048
N_HEADS, N_KV_HEADS, HEAD_DIM = 32, 8, 64
N_CORES = 8
HPC = N_HEADS // N_CORES  # 4 q heads per core
BS = B * S  # 4096 rows
NKT = S // 128  # 16 k tiles per batch
NQC = S // 512  # 4 q chunks per batch
NNT = BS // 512  # 8 projection column blocks
NEG_THRESH = -1e4


def _patched_drain_and_barrier(self, tick_clock, wait_clock):
    # walrus (CoreV3) only accepts one sync-wait on the tile exit drain;
    # split the accumulated waits across single-wait nops.
    nc = self.nc
    drain_inst = nc.sync.drain()
    wait_clock.add_sem_waits(
        drain_inst.ins, tile.ScopedClock({None: tick_clock.global_clock})
    )
    si = drain_inst.ins.sync_info
    sw = list(si.on_wait) if si and si.on_wait else []
    if len(sw) > 1:
        si.on_wait = [sw[0]]
        for w in sw[1:]:
            n2 = nc.sync.nop(nofuse=True)
            if n2.ins.sync_info is None:
                n2.ins.sync_info = mybir.SyncInfo(on_wait=[w], on_update=[])
            else:
                n2.ins.sync_info.on_wait = [w]
    nc.all_engine_barrier()
    assert self.sems is not None
    popped = nc._tile_sem_poison_stack.pop()
    assert popped is self._sem_poison
    nc.clear_and_free_semaphores(list(self.sems.allocated().values()))
    nc.all_engine_barrier()


tile.TileContext._drain_and_barrier = _patched_drain_and_barrier


def _split_multi_waits(nc):
    """walrus (this build) accepts at most one sync-wait per instruction;
    move extra waits onto same-engine nops inserted just before."""
    n_split = 0
    for f in nc.m.functions:
        for blk in f.blocks:
            new_insts = []
            for inst in blk.instructions:
                si = getattr(inst, "sync_info", None)
                if si is not None and si.on_wait and len(si.on_wait) > 1:
                    extra = list(si.on_wait[:-1])
                    si.on_wait = [si.on_wait[-1]]
                    for w in extra:
                        nop = mybir.InstNoOp(
                            name=nc.get_next_instruction_name(), ins=[], outs=[]
                        )
                        nop.engine = inst.engine
                        nop.sync_info = mybir.SyncInfo(on_wait=[w], on_update=[])
                        new_insts.append(nop)
                        n_split += 1
                new_insts.append(inst)
            blk.instructions[:] = new_insts
    return n_split


def build_nc(classes, debug_phase=None):
    """classes[kt][qc] in {'z','n','m'}: mask tile all-zero / all-neg / mixed."""
    nc = bass.Bass("TRN2", target_bir_lowering=False, debug=False, num_devices=N_CORES)

    xT_d = nc.dram_tensor("xT", [DIM, BS], BF16, kind="ExternalInput")
    wq_d = nc.dram_tensor("wq_c", [DIM, HPC * HEAD_DIM], BF16, kind="ExternalInput")
    wk_d = nc.dram_tensor("wk_c", [DIM, HEAD_DIM], BF16, kind="ExternalInput")
    wv_d = nc.dram_tensor("wv_c", [DIM, HEAD_DIM], BF16, kind="ExternalInput")
    wo_d = nc.dram_tensor("wo_c", [HPC * HEAD_DIM, DIM], BF16, kind="ExternalInput")
    maskT_d = nc.dram_tensor("maskT", [S, S], BF16, kind="ExternalInput")
    cosd_d = nc.dram_tensor("cosd", [128, BS], BF16, kind="ExternalInput")
    sind_d = nc.dram_tensor("sind", [128, BS], BF16, kind="ExternalInput")
    perm_d = nc.dram_tensor("perm", [128, 128], BF16, kind="ExternalInput")
    eye64_d = nc.dram_tensor("eye64", [64, 64], BF16, kind="ExternalInput")
    out_d = nc.dram_tensor("out_c", [BS, DIM], BF16, kind="ExternalOutput")
    if debug_phase == "proj":
        dbg_q = nc.dram_tensor("dbg_q", [128, 2 * BS], BF16, kind="ExternalOutput")
        dbg_k = nc.dram_tensor("dbg_k", [128, BS], BF16, kind="ExternalOutput")
        dbg_v = nc.dram_tensor("dbg_v", [128, B * NKT * 65], BF16, kind="ExternalOutput")
    if debug_phase == "attn":
        dbg_at = nc.dram_tensor("dbg_at", [128, 2 * BS], BF16, kind="ExternalOutput")

    with tile.TileContext(nc) as tc:
        with (
            tc.tile_pool(name="persist", bufs=1) as persist,
            tc.tile_pool(name="stream", bufs=2) as stream,
            tc.tile_pool(name="small", bufs=3) as small,
        ):
            # ---- persistent tensors ----
            wq_sb = persist.tile([128, NKT, HPC * HEAD_DIM], BF16, tag="wq")
            wk_sb = persist.tile([128, NKT, HEAD_DIM], BF16, tag="wk")
            wv_sb = persist.tile([128, NKT, HEAD_DIM], BF16, tag="wv")
            wo_sb = persist.tile([128, 2, DIM], BF16, tag="wo")
            perm_sb = persist.tile([128, 128], BF16, tag="perm")
            eye64_sb = persist.tile([64, 64], BF16, tag="eye64")
            ones_sb = persist.tile([128, 64], F32, tag="ones")
            q_sb = persist.tile([128, 2, NNT * 512], BF16, tag="q")  # Q^T
            kT_sb = persist.tile([128, BS], BF16, tag="kT")  # K^T (dup halves)
            v_sb = persist.tile([128, B * NKT, 65], BF16, tag="v")  # [V|1]
            at_sb = persist.tile([128, 2, BS], BF16, tag="at")  # A^T

            nc.sync.dma_start(wq_sb[:], wq_d.rearrange("(t p) m -> p t m", p=128))
            nc.sync.dma_start(wk_sb[:], wk_d.rearrange("(t p) m -> p t m", p=128))
            nc.sync.dma_start(wv_sb[:], wv_d.rearrange("(t p) m -> p t m", p=128))
            nc.sync.dma_start(wo_sb[:], wo_d.rearrange("(t p) m -> p t m", p=128))
            nc.sync.dma_start(perm_sb[:], perm_d[:])
            nc.sync.dma_start(eye64_sb[:], eye64_d[:])
            nc.gpsimd.memset(ones_sb[:], 1.0)
            nc.gpsimd.memset(v_sb[:, :, 64:65], 1.0)

            # ---- phase 1: projections + RoPE, per column block of 512 rows ----
            p1 = tc.tile_pool(name="ps_acc", bufs=4, space="PSUM")
            ps_acc = p1.__enter__()
            p1b = tc.tile_pool(name="ps_swp", bufs=2, space="PSUM")
            ps_swp = p1b.__enter__()
            p1c = tc.tile_pool(name="ps_t", bufs=1, space="PSUM")
            ps_t = p1c.__enter__()
            for nt in range(NNT):
                cs = slice(nt * 512, (nt + 1) * 512)
                xblk = stream.tile([128, NKT, 512], BF16, tag="xblk")
                nc.sync.dma_start(
                    xblk[:], xT_d[:, cs].rearrange("(t p) n -> p t n", p=128)
                )
                cosb = stream.tile([128, 512], BF16, tag="cosb")
                sinb = stream.tile([128, 512], BF16, tag="sinb")
                nc.sync.dma_start(cosb[:], cosd_d[:, cs])
                nc.sync.dma_start(sinb[:], sind_d[:, cs])

                # Q projection: 2 M-tiles of 128 rows (= 2 heads each)
                for mt in range(2):
                    psq = ps_acc.tile([128, 512], F32, tag="acc")
                    for kt in range(NKT):
                        nc.tensor.matmul(
                            psq[:],
                            wq_sb[:, kt, mt * 128 : (mt + 1) * 128],
                            xblk[:, kt, :],
                            start=(kt == 0),
                            stop=(kt == NKT - 1),
                        )
                    q_tmp = small.tile([128, 512], BF16, tag="q_tmp")
                    nc.scalar.mul(q_tmp[:], psq[:], 1.0 / 8.0)
                    psw = ps_swp.tile([128, 512], F32, tag="swp")
                    nc.tensor.matmul(psw[:], perm_sb[:], q_tmp[:])
                    v1 = small.tile([128, 512], BF16, tag="v1")
                    nc.vector.tensor_mul(v1[:], q_tmp[:], cosb[:])
                    v2 = small.tile([128, 512], BF16, tag="v2")
                    nc.vector.tensor_mul(v2[:], psw[:], sinb[:])
                    nc.vector.tensor_add(q_sb[:, mt, cs], v1[:], v2[:])

                # K projection (single 64-row tile)
                psk = ps_acc.tile([64, 512], F32, tag="acc")
                for kt in range(NKT):
                    nc.tensor.matmul(
                        psk[:],
                        wk_sb[:, kt, :],
                        xblk[:, kt, :],
                        start=(kt == 0),
                        stop=(kt == NKT - 1),
                    )
                k_tmp = small.tile([64, 512], BF16, tag="k_tmp")
                nc.scalar.copy(k_tmp[:], psk[:])
                pskw = ps_swp.tile([64, 512], F32, tag="swp")
                nc.tensor.matmul(pskw[:], perm_sb[0:64, 0:64], k_tmp[:])
                kv1 = small.tile([64, 512], BF16, tag="kv1")
                nc.vector.tensor_mul(kv1[:], k_tmp[:], cosb[0:64, :])
                kv2 = small.tile([64, 512], BF16, tag="kv2")
                nc.vector.tensor_mul(kv2[:], pskw[:], sinb[0:64, :])
                nc.vector.tensor_add(kT_sb[0:64, cs], kv1[:], kv2[:])
                # duplicate K^T into partitions 64..127 (so odd q-heads can
                # use it as lhsT at their partition base)
                nc.gpsimd.dma_start(kT_sb[64:128, cs], kT_sb[0:64, cs])

                # V projection -> V^T [64, 512], then transpose to natural V
                psv = ps_acc.tile([64, 512], F32, tag="acc")
                for kt in range(NKT):
                    nc.tensor.matmul(
                        psv[:],
                        wv_sb[:, kt, :],
                        xblk[:, kt, :],
                        start=(kt == 0),
                        stop=(kt == NKT - 1),
                    )
                v_tmp = small.tile([64, 512], BF16, tag="v_tmp")
                nc.scalar.copy(v_tmp[:], psv[:])
                for j in range(4):
                    pst = ps_t.tile([128, 64], BF16, tag="pst")
                    nc.tensor.transpose(
                        pst[:], v_tmp[:, j * 128 : (j + 1) * 128], eye64_sb[:]
                    )
                    rc = nt * 4 + j
                    nc.scalar.copy(v_sb[:, rc, 0:64], pst[:])

            p1c.__exit__(None, None, None)
            p1b.__exit__(None, None, None)
            p1.__exit__(None, None, None)

            if debug_phase == "proj":
                nc.sync.dma_start(dbg_q[:], q_sb[:].rearrange("p a b -> p (a b)"))
                nc.sync.dma_start(dbg_k[:], kT_sb[:])
                nc.sync.dma_start(dbg_v[:], v_sb[:].rearrange("p a b -> p (a b)"))

            # ---- phase 2+3 fused: attention + wo slice per (batch, q chunk) ----
            p2 = tc.tile_pool(name="ps_s", bufs=3, space="PSUM")
            ps_s = p2.__enter__()
            p2b = tc.tile_pool(name="ps_o", bufs=2, space="PSUM")
            ps_o = p2b.__enter__()
            p2c = tc.tile_pool(name="ps_b", bufs=1, space="PSUM")
            ps_b = p2c.__enter__()
            p3 = tc.tile_pool(name="ps_d", bufs=2, space="PSUM")
            ps_d = p3.__enter__()
            for b in range(B) if debug_phase != "proj" else []:
                for qc in range(NQC):
                    qs = slice(b * S + qc * 512, b * S + (qc + 1) * 512)
                    acts = [kt for kt in range(NKT) if classes[kt][qc] != "n"]
                    assert acts, "fully-masked q chunk unsupported"
                    for h in range(HPC):
                        hb = (h % 2) * 64
                        mt = h // 2
                        pso = ps_o.tile([65, 512], F32, tag="pso")
                        for kt in acts:
                            ks = slice(b * S + kt * 128, b * S + (kt + 1) * 128)
                            pss = ps_s.tile([128, 512], F32, tag="pss")
                            nc.tensor.matmul(
                                pss[:],
                                kT_sb[hb : hb + 64, ks],
                                q_sb[hb : hb + 64, mt, qs],
                            )
                            p_t = small.tile([128, 512], BF16, tag="p_t")
                            if classes[kt][qc] == "z":
                                nc.scalar.activation(
                                    p_t[:], pss[:], mybir.ActivationFunctionType.Exp
                                )
                            else:
                                mk = small.tile([128, 512], BF16, tag="mk")
                                nc.gpsimd.dma_start(
                                    mk[:],
                                    maskT_d[
                                        kt * 128 : (kt + 1) * 128,
                                        qc * 512 : (qc + 1) * 512,
                                    ],
                                )
                                s2 = small.tile([128, 512], BF16, tag="s2")
                                nc.vector.tensor_add(s2[:], pss[:], mk[:])
                                nc.scalar.activation(
                                    p_t[:], s2[:], mybir.ActivationFunctionType.Exp
                                )
                            nc.tensor.matmul(
                                pso[:],
                                v_sb[:, b * NKT + kt, :],
                                p_t[:],
                                start=(kt == acts[0]),
                                stop=(kt == acts[-1]),
                            )
                        # normalize: rows 0..63 = unnormalized O^T, row 64 = denom
                        lg = small.tile([65, 512], F32, tag="lg")
                        nc.scalar.activation(
                            lg[64:65, :], pso[64:65, :],
                            mybir.ActivationFunctionType.Ln,
                        )
                        rec = small.tile([65, 512], F32, tag="rec")
                        nc.scalar.activation(
                            rec[64:65, :], lg[64:65, :],
                            mybir.ActivationFunctionType.Exp, scale=-1.0,
                        )
                        psb2 = ps_b.tile([64, 512], F32, tag="psb2")
                        nc.tensor.matmul(psb2[:], ones_sb[64:65, :], rec[64:65, :])
                        bc = small.tile([64, 512], BF16, tag="bc")
                        nc.scalar.copy(bc[:], psb2[:])
                        if hb == 0:
                            nc.vector.tensor_mul(
                                at_sb[0:64, mt, qs], pso[0:64, :], bc[:]
                            )
                        else:
                            att = small.tile([64, 512], BF16, tag="att")
                            nc.vector.tensor_mul(att[:], pso[0:64, :], bc[:])
                            nc.gpsimd.dma_start(at_sb[64:128, mt, qs], att[:])
                    if debug_phase == "attn":
                        continue
                    # wo slice for these 512 rows (4 row-tiles of 128)
                    for j in range(4):
                        mt2 = (b * S + qc * 512) // 128 + j
                        for ntc in range(4):
                            psd = ps_d.tile([128, 512], F32, tag="psd")
                            for ch in range(2):
                                nc.tensor.matmul(
                                    psd[:],
                                    at_sb[:, ch, mt2 * 128 : (mt2 + 1) * 128],
                                    wo_sb[:, ch, ntc * 512 : (ntc + 1) * 512],
                                    start=(ch == 0),
                                    stop=(ch == 1),
                                )
                            ot = small.tile([128, 512], BF16, tag="ot")
                            nc.any.tensor_copy(ot[:], psd[:])
                            nc.gpsimd.dma_start(
                                out_d[
                                    mt2 * 128 : (mt2 + 1) * 128,
                                    ntc * 512 : (ntc + 1) * 512,
                                ],
                                ot[:],
                            )

            if debug_phase == "attn":
                nc.sync.dma_start(dbg_at[:], at_sb[:].rearrange("p a b -> p (a b)"))
            p3.__exit__(None, None, None)
            p2c.__exit__(None, None, None)
            p2b.__exit__(None, None, None)
            p2.__exit__(None, None, None)
    _split_multi_waits(nc)
    return nc


_NC_CACHE = {}


def _classify_mask(mask):
    """Per (kt, qc) tile class from the [S, S] additive mask ([q, k])."""
    classes = []
    for kt in range(NKT):
        row = []
        for qc in range(NQC):
            sub = mask[qc * 512 : (qc + 1) * 512, kt * 128 : (kt + 1) * 128]
            if np.all(sub == 0.0):
                row.append("z")
            elif np.all(sub <= NEG_THRESH):
                row.append("n")
            else:
                row.append("m")
        classes.append(row)
    # every q row must keep at least one active k tile
    for qc in range(NQC):
        if all(classes[kt][qc] == "n" for kt in range(NKT)):
            for kt in range(NKT):
                if classes[kt][qc] == "n":
                    classes[kt][qc] = "m"
    return classes


def _prep_inputs(x, freqs_cos, freqs_sin, mask, wq, wk, wv, wo):
    bf = ml_dtypes.bfloat16
    x2 = np.ascontiguousarray(np.asarray(x, dtype=np.float32).reshape(BS, DIM))
    xT = np.ascontiguousarray(x2.T).astype(bf)
    maskT = np.ascontiguousarray(np.asarray(mask, dtype=np.float32).T).astype(bf)

    cos = np.asarray(freqs_cos, dtype=np.float32)  # [S, 32]
    sin = np.asarray(freqs_sin, dtype=np.float32)
    # cosd[d, b*S+s] = cos[s, (d%64)//2]; sind alternates -sin/+sin
    d = np.arange(128)
    pair = (d % 64) // 2
    cosd = cos[:, pair].T  # [128, S]
    sgn = np.where(d % 2 == 0, -1.0, 1.0).astype(np.float32)
    sind = sin[:, pair].T * sgn[:, None]
    cosd = np.ascontiguousarray(np.tile(cosd, (1, B))).astype(bf)
    sind = np.ascontiguousarray(np.tile(sind, (1, B))).astype(bf)

    perm = np.zeros((128, 128), dtype=np.float32)
    idx = np.arange(128)
    perm[idx ^ 1, idx] = 1.0
    perm = perm.astype(bf)
    eye64 = np.eye(64, dtype=np.float32).astype(bf)

    wq = np.asarray(wq, dtype=np.float32)
    wk = np.asarray(wk, dtype=np.float32)
    wv = np.asarray(wv, dtype=np.float32)
    wo = np.asarray(wo, dtype=np.float32)

    in_maps = []
    for c in range(N_CORES):
        hs = slice(c * HPC * HEAD_DIM, (c + 1) * HPC * HEAD_DIM)
        ks = slice(c * HEAD_DIM, (c + 1) * HEAD_DIM)
        in_maps.append(
            {
                "xT": xT,
                "wq_c": np.ascontiguousarray(wq[:, hs]).astype(bf),
                "wk_c": np.ascontiguousarray(wk[:, ks]).astype(bf),
                "wv_c": np.ascontiguousarray(wv[:, ks]).astype(bf),
                "wo_c": np.ascontiguousarray(wo[hs, :]).astype(bf),
                "maskT": maskT,
                "cosd": cosd,
                "sind": sind,
                "perm": perm,
                "eye64": eye64,
            }
        )
    return in_maps


def kernel(x, freqs_cos, freqs_sin, mask, wq, wk, wv, wo, _trace=False):
    classes = _classify_mask(np.asarray(mask, dtype=np.float32))
    key = tuple(tuple(r) for r in classes)
    if key not in _NC_CACHE:
        _NC_CACHE[key] = build_nc(classes)
    nc = _NC_CACHE[key]
    in_maps = _prep_inputs(x, freqs_cos, freqs_sin, mask, wq, wk, wv, wo)
    res = run_bass_kernel_spmd(
        nc, in_maps, core_ids=list(range(N_CORES)), trace=_trace
    )
    out = np.zeros((BS, DIM), dtype=np.float32)
    for c in range(N_CORES):
        out += np.asarray(res.results[c]["out_c"], dtype=np.float32)
    out = out.reshape(B, S, DIM)
    if _trace:
        kernel._last_exec_time_ns = res.exec_time_ns
        kernel._last_profile_json = res.profile_json
    return out
